# revision 22
# baseline (speedup 1.0000x reference)
"""DINN forward kernel for Trainium2 (Bass/Tile), batch-sharded across 8 NeuronCores.

Reference computation (B=16384, D=512):
    gates  = sigmoid(x @ W.T + b)                       # [B, D]
    linear = sum(gates * x, axis=1)                     # [B]
    quad   = sum_{i<j} iw_ij * x_i * x_j                # [B]
    out    = sigmoid(linear + quad)[:, None]            # [B, 1]

Data-parallel sharding: x is split along the batch across the 8 cores;
W, b and the (strictly upper-triangular) interaction matrix U built from iw
are replicated. No collectives are needed in the forward pass.

Per-core kernel (batch shard of 2048 rows, 4 batch tiles of 512), all matmuls
in the "transposed" orientation with the contraction dim D on SBUF partitions
(host pre-transposes x -> xT).  Per output chunk m (128 of the 512 d's):
    G^T[m,b] = sum_k Wt[k,m] xT[k,b]     2 fp8 DoubleRow matmuls (0.5 cy/row;
                                          W is pre-scaled by 8 on the host to
                                          dodge fp8 subnormals, undone by the
                                          ACT scale)
    T^T[m,b] = sum_{k<=m} U[k,m] xT[k,b]  m+1 fp16 matmuls (1 cy/row).
                 U strictly upper -> 6/16 blocks skipped; the 10 nonzero
                 128x128 blocks are packed host-side (0.3 MB DMA)
    sig_m = sigmoid(G^T/8 + b_m)  ACT, psum -> fp16 sbuf
    s2_m  = sig_m + T^T_m         DVE, (fp16 + f32 psum) -> fp16
    p_m   = s2_m * xT_m           DVE all-fp16 (4x mode, ~194 ns)
    r    += ones^T @ p_m          PE fp16 matmul accumulating [1,512] in PSUM
The ones-matmuls (and each tile's ACT drain + DMA) are deferred three
m-groups so the PE never waits on the ACT->DVE chain.  The kernel returns
log-odds; the final sigmoid runs on host in float64.

Precision budget (log-odds scale ~N(0, 362), tolerance rel 2e-2): fp8 gates
give linear-part error ~0.4; fp16 x/U give quad error ~0.15; both only
perturb the ~1% of rows near the decision boundary.  Measured rel err ~4e-3,
5x under the gate.  All inputs are DMA'd once: x as fp16 + fp8, W as fp8,
U packed fp16 -> 3.6 MB per core total.
"""
import sys

if "/opt/trn_rl_repo" not in sys.path:
    sys.path.insert(0, "/opt/trn_rl_repo")

import numpy as np
from ml_dtypes import float8_e4m3

import concourse.tile as tile
from concourse import bacc, mybir
from concourse.bass_utils import run_bass_kernel_spmd

B, D = 16384, 512
NCORES = 8
BC = B // NCORES            # 2048 rows per core
# batch tiles per core: narrow tail tiles shorten the end-of-kernel
# ACT->DVE->ones-matmul drain chain
TILES = [(0, 512), (512, 512), (1024, 512), (1536, 256), (1792, 256)]
NBT = len(TILES)
NK = D // 128               # 4 contraction chunks

# packed strictly-upper-triangular U: nonzero 128x128 blocks (k <= m) in
# per-m usage order
UBLOCKS = [(m, k) for m in range(NK) for k in range(m + 1)]
UIDX = {mk: i for i, mk in enumerate(UBLOCKS)}
NUB = len(UBLOCKS)          # 10

WSCALE = 8.0                # host pre-scale of W for fp8 range

f32 = mybir.dt.float32
f16 = mybir.dt.float16
f8 = mybir.dt.float8e4
AF = mybir.ActivationFunctionType
DR = mybir.MatmulPerfMode.DoubleRow

_CACHE = {}


def _build():
    nc = bacc.Bacc("TRN2", target_bir_lowering=False, debug=False,
                   num_devices=NCORES)

    d_x16 = nc.declare_dram_parameter("x16", [D, BC], f16, isOutput=False)
    d_xT8 = nc.declare_dram_parameter("xT8", [D, BC], f8, isOutput=False)
    d_W8 = nc.declare_dram_parameter("W8", [D, D], f8, isOutput=False)
    d_Upk = nc.declare_dram_parameter("Upk", [128, NUB * 128], f16,
                                      isOutput=False)
    d_bias = nc.declare_dram_parameter("bias", [D], f32, isOutput=False)
    d_ones = nc.declare_dram_parameter("ones16", [128, 1], f16, isOutput=False)
    d_out = nc.declare_dram_parameter("out", [1, BC], f32, isOutput=True)

    rearr = lambda ap: ap.rearrange("(c p) n -> p c n", p=128)

    with tile.TileContext(nc) as tc:
        with tc.tile_pool(name="const", bufs=1) as const, \
             tc.tile_pool(name="xin", bufs=2) as xin, \
             tc.tile_pool(name="elt", bufs=3) as elt, \
             tc.tile_pool(name="pel", bufs=4) as pel, \
             tc.tile_pool(name="pg", bufs=3, space="PSUM") as pg, \
             tc.tile_pool(name="ptp", bufs=3, space="PSUM") as ptp, \
             tc.tile_pool(name="pop", bufs=2, space="PSUM") as pop:

            # ---- weights / constants (loaded once, replicated per core) ----
            W8_sb = const.tile([128, NK, D], f8, tag="w8")
            Upk_sb = const.tile([128, NUB, 128], f16, tag="upk")
            bias_sb = const.tile([128, NK], f32, tag="bias")
            ones_sb = const.tile([128, 1], f16, tag="ones")

            x16_r = rearr(d_x16[:, :])
            xT8_r = rearr(d_xT8[:, :])
            W8_d = rearr(d_W8[:, :])
            Upk_d = d_Upk[:, :].rearrange("p (i n) -> p i n", n=128)

            # PE warm-up: ~24 tiny fp16 matmuls on a zeroed tile keep the PE
            # continuously busy from ~0.4us so its p-state ramp (full clock
            # after 3us of busy) completes before the real matmuls arrive
            wz = const.tile([128, 128], f16, tag="wz")
            nc.vector.memset(wz[:, :], 0.0)
            wps = pg.tile([128, 512], f32, tag="pg")
            for _ in range(24):
                nc.tensor.matmul(wps[0:1, 0:128], wz[:, 0:1], wz[:, :],
                                 start=True, stop=True)

            # prologue transfers in need order: the fp8 gates operands are
            # tiny so the PE starts ~3us in; the fp16 x tile and packed U
            # stream in while the gates run
            x80 = xin.tile([128, NK, 512], f8, tag="x8")
            nc.sync.dma_start(out=x80[:, 0:2, :], in_=xT8_r[:, 0:2, 0:512])
            nc.sync.dma_start(out=W8_sb, in_=W8_d)
            nc.sync.dma_start(out=x80[:, 2:NK, :], in_=xT8_r[:, 2:NK, 0:512])
            nc.sync.dma_start(
                out=bias_sb, in_=d_bias[:].rearrange("(c p) -> p c", p=128))
            x160 = xin.tile([128, NK, 512], f16, tag="x16")
            nc.sync.dma_start(out=Upk_sb[:, 0:3, :], in_=Upk_d[:, 0:3, :])
            nc.sync.dma_start(out=x160[:, 0:2, :], in_=x16_r[:, 0:2, 0:512])
            nc.sync.dma_start(out=ones_sb, in_=d_ones[:, :])
            nc.sync.dma_start(out=Upk_sb[:, 3:6, :], in_=Upk_d[:, 3:6, :])
            nc.sync.dma_start(out=x160[:, 2:NK, :], in_=x16_r[:, 2:NK, 0:512])
            nc.sync.dma_start(out=Upk_sb[:, 6:NUB, :], in_=Upk_d[:, 6:NUB, :])

            # ones-matmuls deferred >= 3 m-groups behind the producing group
            # so the PE never waits on the ACT->DVE chain
            pending = []        # (group_idx, emit_fn)

            def flush(now):
                while pending and pending[0][0] <= now - 3:
                    pending.pop(0)[1]()

            for b0, (bst, bw) in enumerate(TILES):
                bsl = slice(bst, bst + bw)
                if b0 == 0:
                    x8, x16 = x80, x160
                else:
                    x8 = xin.tile([128, NK, 512], f8, tag="x8")
                    nc.sync.dma_start(out=x8[:, :, 0:bw], in_=xT8_r[:, :, bsl])
                    x16 = xin.tile([128, NK, 512], f16, tag="x16")
                    nc.sync.dma_start(out=x16[:, :, 0:bw], in_=x16_r[:, :, bsl])

                po_t = pop.tile([1, 512], f32, tag="po")
                po = po_t[0:1, 0:bw]

                for m in range(NK):
                    g = b0 * NK + m
                    msl = slice(m * 128, (m + 1) * 128)
                    # gates: G^T chunk, 2 fp8 DoubleRow matmuls (K=256 each)
                    psum_gf = pg.tile([128, 512], f32, tag="pg")
                    psum_g = psum_gf[:, 0:bw]
                    for kc in range(2):
                        nc.tensor.matmul(
                            psum_g, W8_sb[:, 2 * kc:2 * kc + 2, msl],
                            x8[:, 2 * kc:2 * kc + 2, 0:bw],
                            start=(kc == 0), stop=(kc == 1), perf_mode=DR)
                    # quad: T^T chunk from the packed nonzero fp16 blocks
                    ptm_f = ptp.tile([128, 512], f32, tag="pt")
                    ptm = ptm_f[:, 0:bw]
                    for k in range(m + 1):
                        nc.tensor.matmul(
                            ptm, Upk_sb[:, UIDX[(m, k)], :], x16[:, k, 0:bw],
                            start=(k == 0), stop=(k == m))
                    flush(g)

                    sig_f = elt.tile([128, 512], f16, tag="sig")
                    sig = sig_f[:, 0:bw]
                    nc.scalar.activation(sig, psum_g, AF.Sigmoid,
                                         bias=bias_sb[:, m:m + 1],
                                         scale=1.0 / WSCALE)
                    s2_f = elt.tile([128, 512], f16, tag="s2")
                    s2 = s2_f[:, 0:bw]
                    nc.vector.tensor_add(s2, sig, ptm)
                    p_mf = pel.tile([128, 512], f16, tag="p")
                    p_m = p_mf[:, 0:bw]
                    nc.vector.tensor_mul(p_m, s2, x16[:, m, 0:bw])

                    def emit(po=po, p_m=p_m, m=m, bsl=bsl, bw=bw):
                        nc.tensor.matmul(
                            po, ones_sb, p_m,
                            start=(m == 0), stop=(m == NK - 1))
                        if m == NK - 1:
                            # tile finished: [1, bw] log-odds PSUM -> SBUF
                            # (ACT copy) -> DMA to DRAM.  Emitted HERE, after
                            # the stop matmul, so the emission-order dependency
                            # tracking sees all 4 accumulating matmuls.
                            ot_f = elt.tile([1, 512], f32, tag="ot")
                            ot = ot_f[0:1, 0:bw]
                            nc.scalar.activation(ot, po, AF.Copy)
                            nc.sync.dma_start(out=d_out[0:1, bsl], in_=ot)
                    pending.append((g, emit))

                if b0 == NBT - 1:
                    while pending:
                        pending.pop(0)[1]()

    nc.compile()
    return nc


def kernel(x, W, b, iw):
    x = np.asarray(x, np.float32)
    W = np.asarray(W, np.float32)
    b = np.asarray(b, np.float32)
    iw = np.asarray(iw, np.float32)

    # host prep: strictly upper-triangular U from iw (row-major i<j order),
    # pre-transposed operands so the contraction dim lands on SBUF partitions
    U = np.zeros((D, D), np.float32)
    iu, ju = np.triu_indices(D, k=1)
    U[iu, ju] = iw
    # pack the 10 nonzero 128x128 blocks of U, contraction dim on partitions
    Upk = np.empty((128, NUB * 128), np.float16)
    for i, (m, k) in enumerate(UBLOCKS):
        Upk[:, i * 128:(i + 1) * 128] = U[k * 128:(k + 1) * 128,
                                          m * 128:(m + 1) * 128]
    W8 = np.ascontiguousarray(W.T * WSCALE).astype(float8_e4m3)
    xT = x.T                                 # [D, B] view
    xT8 = xT.astype(float8_e4m3)
    xT16 = xT.astype(np.float16)

    shared = {"W8": W8, "Upk": Upk, "bias": b,
              "ones16": np.ones((128, 1), np.float16)}
    in_maps = []
    for c in range(NCORES):
        mp = dict(shared)
        mp["x16"] = np.ascontiguousarray(xT16[:, c * BC:(c + 1) * BC])
        mp["xT8"] = np.ascontiguousarray(xT8[:, c * BC:(c + 1) * BC])
        in_maps.append(mp)

    if "nc" not in _CACHE:
        _CACHE["nc"] = _build()
    nc = _CACHE["nc"]

    res = run_bass_kernel_spmd(nc, in_maps, list(range(NCORES)))
    lo = np.concatenate(
        [res.results[c]["out"][0] for c in range(NCORES)]).astype(np.float64)
    out = 1.0 / (1.0 + np.exp(-np.clip(lo, -708.0, 708.0)))
    return out.reshape(B, 1).astype(np.float32)
